# revision 1
# baseline (speedup 1.0000x reference)
"""Block-sparse attention kernel for Trainium2 (8 NeuronCores).

Problem: B=2, S=2048, H=16, Dqk=Dv=64, 64x64 block mask (30% + forced diag),
AND causal. out = softmax(mask(QK^T/8)) @ V.

Strategy
--------
- Shard the 32 (batch, head) pairs across 8 cores, 4 heads per core.
- Each core gets its OWN Bass program with the sparse block schedule baked in
  from its heads' block masks (compiled at call time, run concurrently on the
  8 axon devices).
- Per head, scores are computed TRANSPOSED (S^T[k, q]) so that P^T = exp(S^T)
  lands in SBUF in exactly the layout PV needs (k on partitions) — no on-chip
  transposes anywhere:
    * host supplies Q^T and K^T as [64(d), 2048(s)] fp16, V as [128, 16*65]
      fp16 "v-pair" tiles [V[2t]; V[2t+1]] with a ones column (col 64).
    * k-blocks are processed in pairs (2t, 2t+1) = 128 partitions.
    * QK: matmul(lhsT=K^T pair [64,128], rhs=Q^T qb-run [64,64n]) -> PSUM.
    * exp: one ACT op per ~1024 PSUM columns (scale=1/8 fused), fp16 out.
    * fixups (DVE): zero inactive 64x64 half-blocks, multiply causal triangle
      into diagonal blocks.
    * PV: matmul(lhsT=[V|1] pair [128,65], rhs=P^T run) accumulating O^T[65,
      2048] in PSUM across k-pairs (start/stop on first/last touch per qb).
    * O^T (unnormalized, with row 64 = softmax denominator l) is copied to
      SBUF and DMA'd out; the host divides and transposes back.
- Softmax uses no running max: inputs are N(0,1) so scores/8 stay in a range
  where exp() is safely finite in fp32 (exp(~7) ~ 1e3).
"""

import threading
from contextlib import ExitStack

import numpy as np

import concourse.bass as bass
import concourse.tile as tile
from concourse import mybir
from concourse.bass_utils import run_bass_kernel_spmd
from concourse.vector_clock import ScopedClock

# ----------------------------------------------------------------------------
# Workaround: the installed walrus rejects instructions with more than one
# sync wait. Tile's kernel-tail drain attaches every outstanding clock sem to
# one Drain instruction; split them one wait per Drain.
# ----------------------------------------------------------------------------


def _split_drain_and_barrier(self, tick_clock, wait_clock):
    nc = self.nc
    drain_inst = nc.sync.drain()
    wait_clock.add_sem_waits(
        drain_inst.ins, ScopedClock({None: tick_clock.global_clock})
    )
    si = drain_inst.ins.sync_info
    waits = list(si.on_wait) if si is not None else []
    if len(waits) > 1:
        drain_inst.ins.sync_info = mybir.SyncInfo(
            on_wait=waits[:1], on_update=list(si.on_update)
        )
        for w in waits[1:]:
            d2 = nc.sync.drain()
            d2.ins.sync_info = mybir.SyncInfo(on_wait=[w], on_update=[])
    nc.all_engine_barrier()
    popped = nc._tile_sem_poison_stack.pop()
    assert popped is self._sem_poison
    nc.clear_and_free_semaphores(list(self.sems.allocated().values()))
    nc.all_engine_barrier()


tile.TileContext._drain_and_barrier = _split_drain_and_barrier

# Walrus is invoked with --enable-ldw-opt=false by default; our kernel issues
# many small self-loading matmuls whose LDWEIGHTS dominate PE time, so flip it
# (opt-out via BASS_NO_LDW_OPT=1).
import os as _os

from concourse import bass_utils as _bass_utils

_orig_run_command = _bass_utils.run_command


def _run_command_ldw_opt(argv, **kwargs):
    if _os.environ.get("BASS_LDW_OPT"):
        argv = [
            a.replace("--enable-ldw-opt=false", "--enable-ldw-opt=true")
            if isinstance(a, str)
            else a
            for a in argv
        ]
    return _orig_run_command(argv, **kwargs)


_bass_utils.run_command = _run_command_ldw_opt


def _dedup_ldweights(nc):
    """Post-scheduling peephole: consecutive matmuls sharing identical weights
    reload them every time (self-loading InstMatmult); LDWEIGHTS dominates PE
    time for small-N matmuls. Rewrite each same-weights chain as one explicit
    InstLdweights followed by non-self-loading matmuls (ldweights=False).

    The chain head's sem waits move onto the InstLdweights so the weight read
    cannot overtake its producers. Run BEFORE _split_multi_waits.
    """
    for fn in nc.m.functions:
        for bb in fn.blocks:
            out = []
            last_key = None
            changed = False
            for inst in bb.instructions:
                if isinstance(inst, mybir.InstMatmult) and not inst.is_transpose:
                    w = inst.ins[1]
                    key = (
                        getattr(w, "memref", None),
                        w.offset,
                        str(w.ap),
                        str(getattr(w, "dtype", None)),
                        inst.tile_position,
                        inst.perf_mode,
                    )
                    if key == last_key:
                        inst.ldweights = False
                        changed = True
                        out.append(inst)
                        continue
                    si = inst.sync_info
                    ldw = mybir.InstLdweights(
                        name=nc.get_next_instruction_name(),
                        engine=inst.engine,
                        ins=[w],
                        outs=[],
                        perf_mode=inst.perf_mode,
                        is_transpose=inst.is_transpose,
                        tile_position=inst.tile_position,
                        tile_size=inst.tile_size,
                        sync_info=mybir.SyncInfo(
                            on_wait=list(si.on_wait) if si else [], on_update=[]
                        ),
                    )
                    inst.sync_info = mybir.SyncInfo(
                        on_wait=[], on_update=list(si.on_update) if si else []
                    )
                    inst.ldweights = False
                    out.extend([ldw, inst])
                    last_key = key
                    changed = True
                elif isinstance(inst, (mybir.InstNoOp, mybir.InstEventSemaphore)):
                    out.append(inst)  # doesn't disturb loaded weights
                else:
                    if inst.engine == mybir.EngineType.PE:
                        last_key = None
                    out.append(inst)
            if changed:
                bb.instructions = out


def _split_multi_waits(nc):
    """Hoist extra sync waits onto same-engine NOPs (walrus: 1 wait/inst)."""
    for fn in nc.m.functions:
        for bb in fn.blocks:
            out = []
            changed = False
            for inst in bb.instructions:
                si = inst.sync_info
                if si is not None and len(si.on_wait) > 1:
                    waits = list(si.on_wait)
                    for w in waits[:-1]:
                        out.append(
                            mybir.InstNoOp(
                                name=nc.get_next_instruction_name(),
                                engine=inst.engine,
                                sync_info=mybir.SyncInfo(on_wait=[w], on_update=[]),
                                bass_nofuse=True,
                            )
                        )
                    inst.sync_info = mybir.SyncInfo(
                        on_wait=[waits[-1]], on_update=list(si.on_update)
                    )
                    changed = True
                out.append(inst)
            if changed:
                bb.instructions = out

# ---------------------------------------------------------------------------
# Problem constants (hardcoded per the task contract)
# ---------------------------------------------------------------------------
B, S, H, D = 2, 2048, 16, 64
NB = 32  # number of 64-wide blocks along S
N_CORES = 8
HPC = 4  # heads (flat b*H+h) per core
CHUNK = 16  # score col-blocks per PSUM chunk (16*64 = 1024 fp32 = 2 banks)
F16 = mybir.dt.float16
F32 = mybir.dt.float32


def _match_pairs(mask):
    """Pair up the 32 k-blocks to maximize overlap of their active-q sets
    (greedy max-weight matching). Overlapping pairs make dense (dual) score
    columns, shrinking the union column count that drives QK/exp/PV work."""
    act = {
        kb: frozenset(qb for qb in range(kb, NB) if mask[qb, kb]) for kb in range(NB)
    }
    left = set(range(NB))
    pairs = []
    while left:
        best = None
        for i in left:
            for j in left:
                if j <= i:
                    continue
                sc = len(act[i] & act[j])
                if best is None or sc > best[0] or (sc == best[0] and (i, j) < best[1:]):
                    best = (sc, i, j)
        _, i, j = best
        pairs.append((i, j))
        left -= {i, j}
    pairs.sort()
    return pairs


def _head_schedule(mask, pairs, gap=2):
    """Columns of the S^T score layout for one head.

    mask: [32, 32] bool. Active block (qb, kb) requires qb >= kb (block-level
    causal) and mask[qb, kb]. pairs: 16 (kb1, kb2) k-block pairs; pair t forms
    the 128-partition tile [K[kb1]; K[kb2]].

    Emission order is qb-bank-group-major (g = qb//8), then pair-major: PSUM
    start=True clears has_written for the whole destination BANK, so each O^T
    bank is zero-opened once (the only start=True) and PV accumulates with
    start=False in any order.

    Interior qb-gaps of <= `gap` within a (g, t) sequence are bridged with
    fake columns (top=bot=False -> fully zeroed P^T) so QK/PV runs merge into
    fewer, larger matmuls.
    """
    cols = []
    for g in range(NB // 8):
        for t, (kb1, kb2) in enumerate(pairs):
            seq = []
            for qb in range(8 * g, 8 * (g + 1)):
                top = qb >= kb1 and bool(mask[qb, kb1])
                bot = qb >= kb2 and bool(mask[qb, kb2])
                if top or bot:
                    seq.append((qb, top, bot))
            ext = []
            for idx, (qb, top, bot) in enumerate(seq):
                if ext:
                    prev_qb = ext[-1][0]
                    if 1 < qb - prev_qb <= gap + 1:
                        for fqb in range(prev_qb + 1, qb):
                            ext.append((fqb, False, False))
                ext.append((qb, top, bot))
            for qb, top, bot in ext:
                cols.append(
                    {
                        "t": t,
                        "qb": qb,
                        "top": top,
                        "bot": bot,
                        "kb1": kb1,
                        "kb2": kb2,
                        "g": g,
                    }
                )
    return cols


def _is_diag_pair(c, nxt):
    """col c = (qb==kb1, top tri) directly followed by its partner col
    (qb==kb2==qb+1, bot tri) of the same pair -> one [128,128] pattern op."""
    return (
        c["qb"] == c["kb1"]
        and c["top"]
        and nxt is not None
        and nxt["t"] == c["t"]
        and nxt["qb"] == c["qb"] + 1
        and nxt["qb"] == nxt["kb2"]
        and nxt["bot"]
    )


def _runs(chunk, key_consecutive, bank_of, flags=None):
    """Split a chunk (list of (idx, col)) into affine matmul runs.

    key_consecutive(prev, cur) -> bool: can cur extend the run?
    bank_of(idx, col) -> int: PSUM bank id of the run target; run must stay in
      one bank.
    flags(col) -> hashable: must be uniform within a run (or None).
    """
    runs = []
    cur = []
    for item in chunk:
        if cur:
            _, pc = cur[-1]
            _, cc = item
            ok = (
                key_consecutive(pc, cc)
                and bank_of(*item) == bank_of(*cur[0])
                and (flags is None or flags(cc) == flags(pc))
            )
            if ok:
                cur.append(item)
                continue
            runs.append(cur)
        cur = [item]
    if cur:
        runs.append(cur)
    return runs


def _chunks_of(cols):
    """Cut cols into chunks of <= CHUNK, never splitting a diagonal pair."""
    chunks = []
    cur = []
    i = 0
    while i < len(cols):
        nxt = cols[i + 1] if i + 1 < len(cols) else None
        take = 2 if _is_diag_pair(cols[i], nxt) else 1
        if len(cur) + take > CHUNK:
            chunks.append(cur)
            cur = []
        cur.extend(cols[i : i + take])
        i += take
    if cur:
        chunks.append(cur)
    return chunks


def build_program(schedules):
    """Build the Bass program for one core.

    schedules: list of HPC dicts {"pairs": [(kb1, kb2)]*16, "cols": [...]}.
    """
    nc = bass.Bass()
    qt = nc.declare_dram_parameter("qt", [HPC, 64, S], F16, isOutput=False)
    kt = nc.declare_dram_parameter("kt", [HPC, 64, S], F16, isOutput=False)
    va = nc.declare_dram_parameter("va", [HPC, 128, 16 * 65], F16, isOutput=False)
    tri = nc.declare_dram_parameter("tri", [128, 64], F16, isOutput=False)
    pats = nc.declare_dram_parameter("pats", [128, 256], F16, isOutput=False)
    ot = nc.declare_dram_parameter("ot", [HPC, 65, S], F32, isOutput=True)

    with tile.TileContext(nc) as tc, ExitStack() as ctx:
        const = ctx.enter_context(tc.tile_pool(name="const", bufs=1))
        pts = ctx.enter_context(tc.tile_pool(name="pts", bufs=3))
        outp = ctx.enter_context(tc.tile_pool(name="outp", bufs=2))
        psS = ctx.enter_context(tc.tile_pool(name="psS", bufs=2, space="PSUM"))
        psO = ctx.enter_context(tc.tile_pool(name="psO", bufs=1, space="PSUM"))

        tri_t = const.tile([128, 64], F16, tag="tri")
        nc.sync.dma_start(out=tri_t[:], in_=tri[:])
        pats_t = const.tile([128, 256], F16, tag="pats")
        nc.sync.dma_start(out=pats_t[:], in_=pats[:])
        zeros = const.tile([128, 512], F16, tag="zeros")
        nc.vector.memset(zeros[:], 0.0)

        # PE warm-up: the HAM clock gate keeps a cold PE at 1.2 GHz; burn
        # ~7 us of dummy matmuls (overlapping the input DMAs) to reach 2.4.
        wps = psS.tile([128, 64 * CHUNK], F32, tag="ps")
        for _ in range(20):
            nc.tensor.matmul(
                wps[:, 0:512],
                lhsT=zeros[:, 0:128],
                rhs=zeros[:, 0:512],
                start=True,
                stop=True,
            )

        qts, kts, vas = [], [], []
        for s in range(HPC):
            qs = const.tile([64, S], F16, tag=f"qt{s}")
            ks = const.tile([64, S], F16, tag=f"kt{s}")
            vs = const.tile([128, 16 * 65], F16, tag=f"va{s}")
            nc.sync.dma_start(out=qs[:], in_=qt[s])
            nc.sync.dma_start(out=ks[:], in_=kt[s])
            nc.sync.dma_start(out=vs[:], in_=va[s])
            qts.append(qs)
            kts.append(ks)
            vas.append(vs)

        for s in range(HPC):
            cols = schedules[s]["cols"]
            oT = psO.tile([128, S], F32, tag="psO")
            # Zero-open each O^T bank (8 q-blocks = 512 fp32 cols) with the
            # group's only start=True matmul; PV then accumulates start=False.
            for g in range(NB // 8):
                nc.tensor.matmul(
                    oT[0:65, 512 * g : 512 * (g + 1)],
                    lhsT=zeros[:, 0:65],
                    rhs=zeros[:, 0:512],
                    start=True,
                    stop=False,
                    skip_group_check=True,
                )
            for chunk_cols in _chunks_of(cols):
                chunk = list(enumerate(chunk_cols))
                L = len(chunk)
                ps = psS.tile([128, 64 * CHUNK], F32, tag="ps")

                # QK: lhsT = K^T pair (fixed per t), rhs = Q^T qb-run.
                qk = _runs(
                    chunk,
                    key_consecutive=lambda p, c: p["t"] == c["t"]
                    and c["qb"] == p["qb"] + 1,
                    bank_of=lambda i, c: i // 8,
                )
                for run in qk:
                    i0, rc = run[0]
                    n = len(run)
                    nc.tensor.matmul(
                        ps[:, 64 * i0 : 64 * (i0 + n)],
                        lhsT=kts[s][:, 128 * rc["t"] : 128 * (rc["t"] + 1)],
                        rhs=qts[s][:, 64 * rc["qb"] : 64 * (rc["qb"] + n)],
                        start=True,
                        stop=True,
                    )

                pt = pts.tile([128, 64 * CHUNK], F16, tag="pt")
                nc.scalar.activation(
                    out=pt[:, : 64 * L],
                    in_=ps[:, : 64 * L],
                    func=mybir.ActivationFunctionType.Exp,
                    scale=0.125,
                )

                # Fixups on P^T: zero inactive halves, causal tri on diagonal.
                # Diagonal pairs (cols qb=2t, 2t+1 adjacent) are handled by ONE
                # tensor_mul against a precomputed [128,128] pattern; remaining
                # dead halves are zeroed with batched memsets.
                need_top = [False] * L  # memset rows 0:64
                need_bot = [False] * L
                i = 0
                while i < L:
                    c = chunk[i][1]
                    if _is_diag_pair(c, chunk[i + 1][1] if i + 1 < L else None):
                        p0 = 0 if chunk[i + 1][1]["top"] else 128
                        nc.vector.tensor_mul(
                            pt[:, 64 * i : 64 * (i + 2)],
                            pt[:, 64 * i : 64 * (i + 2)],
                            pats_t[:, p0 : p0 + 128],
                        )
                        i += 2
                        continue
                    if not c["top"]:
                        need_top[i] = True
                    elif c["qb"] == c["kb1"]:
                        nc.vector.tensor_mul(
                            pt[0:64, 64 * i : 64 * (i + 1)],
                            pt[0:64, 64 * i : 64 * (i + 1)],
                            tri_t[0:64],
                        )
                    if not c["bot"]:
                        need_bot[i] = True
                    elif c["qb"] == c["kb2"]:
                        nc.vector.tensor_mul(
                            pt[64:128, 64 * i : 64 * (i + 1)],
                            pt[64:128, 64 * i : 64 * (i + 1)],
                            tri_t[64:128],
                        )
                    i += 1
                for half, need in ((slice(0, 64), need_top), (slice(64, 128), need_bot)):
                    i = 0
                    while i < L:
                        if need[i]:
                            j = i
                            while j + 1 < L and need[j + 1]:
                                j += 1
                            nc.vector.memset(pt[half, 64 * i : 64 * (j + 1)], 0.0)
                            i = j + 1
                        else:
                            i += 1

                # PV: lhsT = [V|1] pair (fixed per t), rhs = P^T run, out
                # accumulates O^T columns of the run's q-blocks.
                pv = _runs(
                    chunk,
                    key_consecutive=lambda p, c: p["t"] == c["t"]
                    and c["qb"] == p["qb"] + 1,
                    bank_of=lambda i, c: c["qb"] // 8,
                )
                for run in pv:
                    i0, rc = run[0]
                    n = len(run)
                    nc.tensor.matmul(
                        oT[0:65, 64 * rc["qb"] : 64 * (rc["qb"] + n)],
                        lhsT=vas[s][:, 65 * rc["t"] : 65 * (rc["t"] + 1)],
                        rhs=pt[:, 64 * i0 : 64 * (i0 + n)],
                        start=False,
                        stop=True,
                        skip_group_check=True,
                    )

            o_sb = outp.tile([65, S], F32, tag="o")
            nc.vector.tensor_copy(out=o_sb[:], in_=oT[0:65, :])
            nc.sync.dma_start(out=ot[s], in_=o_sb[:])

    _split_multi_waits(nc)
    return nc


def _prep_inputs(q, k, v, schedules):
    """Per-core input arrays keyed as the programs expect."""
    # flat head g = b*H + h
    qt_all = np.ascontiguousarray(
        q.transpose(0, 2, 3, 1).reshape(B * H, D, S).astype(np.float16)
    )
    kt_nat = k.transpose(0, 2, 3, 1).reshape(B * H, D, S).astype(np.float16)
    kt_nat = kt_nat.reshape(B * H, D, NB, 64)
    kt_all = np.empty_like(kt_nat)
    for g in range(B * H):
        order = [kb for p in schedules[g]["pairs"] for kb in p]
        kt_all[g] = kt_nat[g][:, order, :]
    kt_all = np.ascontiguousarray(kt_all.reshape(B * H, D, S))
    v_aug = np.concatenate([v, np.ones((B, S, H, 1), v.dtype)], axis=3)  # [B,S,H,65]
    vb_all = v_aug.transpose(0, 2, 1, 3).reshape(B * H, NB, 64, 65)  # [g, kb, 64, 65]
    # va[g]: per pair t, rows 0:64 = V[kb1] block, rows 64:128 = V[kb2]
    va_all = np.zeros((B * H, 128, 16 * 65), np.float16)
    for g in range(B * H):
        for t, (kb1, kb2) in enumerate(schedules[g]["pairs"]):
            va_all[g, 0:64, 65 * t : 65 * (t + 1)] = vb_all[g, kb1]
            va_all[g, 64:128, 65 * t : 65 * (t + 1)] = vb_all[g, kb2]
    # tri[kl, ql] = 1 where kl <= ql (allowed), both halves
    triu = np.triu(np.ones((64, 64), np.float16))
    tri_full = np.ascontiguousarray(np.concatenate([triu, triu], axis=0))
    # Diagonal-pair patterns [128, 256]: pattern for adjacent cols (qb=2t,
    # qb=2t+1): col 2t = [tri; 0], col 2t+1 = [on_or_off; tri].
    zero = np.zeros((64, 64), np.float16)
    one = np.ones((64, 64), np.float16)
    patA = np.block([[triu, one], [zero, triu]]).astype(np.float16)
    patB = np.block([[triu, zero], [zero, triu]]).astype(np.float16)
    pats_full = np.ascontiguousarray(np.concatenate([patA, patB], axis=1))
    in_maps = []
    for c in range(N_CORES):
        sl = slice(HPC * c, HPC * (c + 1))
        in_maps.append(
            {
                "qt": qt_all[sl],
                "kt": kt_all[sl],
                "va": va_all[sl],
                "tri": tri_full,
                "pats": pats_full,
            }
        )
    return in_maps


def _schedules(block_mask):
    """Per flat head: greedy k-block pairing + column schedule."""
    masks_all = np.asarray(block_mask).reshape(B * H, NB, NB)
    scheds = []
    for g in range(B * H):
        pairs = [(2 * t, 2 * t + 1) for t in range(NB // 2)]
        scheds.append(
            {"pairs": pairs, "cols": _head_schedule(masks_all[g], pairs, gap=0)}
        )
    return scheds


_PROG_CACHE = {}


def _get_programs(block_mask, schedules):
    key = np.asarray(block_mask).tobytes()
    if key not in _PROG_CACHE:
        _PROG_CACHE[key] = [
            build_program(schedules[HPC * c : HPC * (c + 1)]) for c in range(N_CORES)
        ]
    return _PROG_CACHE[key]


def run_cores(ncs, in_maps, trace=False):
    """Run the 8 per-core programs concurrently on the 8 devices."""
    import jax

    devs = jax.devices()
    results = [None] * N_CORES
    errs = [None] * N_CORES

    def _run(c):
        try:
            with jax.default_device(devs[c]):
                r = run_bass_kernel_spmd(
                    ncs[c], [in_maps[c]], core_ids=[0], trace=trace and c == 0
                )
                results[c] = r
        except Exception as e:  # noqa: BLE001
            errs[c] = e

    threads = [threading.Thread(target=_run, args=(c,)) for c in range(N_CORES)]
    for t in threads:
        t.start()
    for t in threads:
        t.join()
    for c, e in enumerate(errs):
        if e is not None:
            raise RuntimeError(f"core {c} failed") from e
    return results


def kernel(q, k, v, block_mask):
    q = np.asarray(q, dtype=np.float32)
    k = np.asarray(k, dtype=np.float32)
    v = np.asarray(v, dtype=np.float32)
    block_mask = np.asarray(block_mask).astype(bool)

    schedules = _schedules(block_mask)
    in_maps = _prep_inputs(q, k, v, schedules)
    ncs = _get_programs(block_mask, schedules)
    results = run_cores(ncs, in_maps)

    out = np.empty((B, S, H, D), np.float32)
    for c in range(N_CORES):
        ot = results[c].results[0]["ot"]  # [HPC, 65, S]
        for s in range(HPC):
            g = HPC * c + s
            b, h = divmod(g, H)
            o_un = ot[s, :D, :]  # [D, S] unnormalized
            l = ot[s, D, :]  # [S]
            out[b, :, h, :] = (o_un / l[None, :]).T
    return out

